# revision 86
# baseline (speedup 1.0000x reference)
"""AVT VQ-VAE encoder forward on 8 Trainium2 NeuronCores.

Data-parallel over the batch dim B=32 (4 clips per core). Each core:
  - computes, per modality (video/audio/text), the [1024, 1024] negated
    distance matrix via fp16 3-pass (Dekker-split) matmuls that reproduce
    XLA-CPU fp32 matmul numerics to ~1e-8,
  - reproduces the reference's exact f32 rounding sequence
    dist = fl(fl(e2+f2) - 2*dot) so the argmin matches the f32 reference,
  - takes per-token argmin (DVE max8/max_index on negated distances),
  - computes softmax(-sqrt(dist)) statistics (ACT ln/exp, fused row-sum)
    and accumulates per-clip mean soft-assignments pH via PE matmul,
  - gathers full codebook rows (indirect DMA) and emits the
    straight-through quantized outputs.
Host assembles shards, computes the tiny [32,32] Lcmcm losses from pH and
the per-clip index modes / equal_num from the argmin indices.
"""

import numpy as np

_B, _T, _D, _M, _NC = 32, 256, 256, 1024, 8
_BL = _B // _NC        # clips per core
_NL = _BL * _T         # tokens per core
_NT = _NL // 128       # 128-token tiles per core
_MODS = ("v", "a", "t")

_CACHE = {}


def _build_bass():
    from contextlib import ExitStack

    import concourse.bacc as bacc
    import concourse.bass as bass
    import concourse.tile as tile
    from concourse import mybir

    f32 = mybir.dt.float32
    f16 = mybir.dt.float16
    bf16 = mybir.dt.bfloat16
    u32 = mybir.dt.uint32
    ALU = mybir.AluOpType
    ACT = mybir.ActivationFunctionType

    nc = bacc.Bacc(trn_type="TRN2", debug=False)

    din, dout = {}, {}
    for x in _MODS:
        din[f"embsp_{x}"] = nc.dram_tensor(f"embsp_{x}", [_D, 2 * _M], f16, kind="ExternalInput").ap()
        din[f"semsp_{x}"] = nc.dram_tensor(f"semsp_{x}", [_D, 2 * _NL], f16, kind="ExternalInput").ap()
        din[f"sem_{x}"] = nc.dram_tensor(f"sem_{x}", [_NL, _D], f32, kind="ExternalInput").ap()
        din[f"e2_{x}"] = nc.dram_tensor(f"e2_{x}", [1, _M], f32, kind="ExternalInput").ap()
        din[f"f2T_{x}"] = nc.dram_tensor(f"f2T_{x}", [128, _NT], f32, kind="ExternalInput").ap()
        dout[f"full_{x}"] = nc.dram_tensor(f"full_{x}", [_NL, 3 * _D], f32, kind="ExternalOutput").ap()
        dout[f"quant_{x}"] = nc.dram_tensor(f"quant_{x}", [_NL, _D], f32, kind="ExternalOutput").ap()
    dout["idx_all"] = nc.dram_tensor("idx_all", [128, 3 * _NT, 8], u32, kind="ExternalOutput").ap()
    dout["pH_all"] = nc.dram_tensor("pH_all", [_BL, 3 * _M], f32, kind="ExternalOutput").ap()
    emb = nc.dram_tensor("emb", [_M, 3 * _D], f32, kind="ExternalInput").ap()

    with tile.TileContext(nc) as tc, ExitStack() as ctx:
        const = ctx.enter_context(tc.tile_pool(name="const", bufs=1))
        work = ctx.enter_context(tc.tile_pool(name="work", bufs=2))
        small = ctx.enter_context(tc.tile_pool(name="small", bufs=4))
        psd = ctx.enter_context(tc.tile_pool(name="psd", bufs=2, space="PSUM"))
        psh = ctx.enter_context(tc.tile_pool(name="psh", bufs=2, space="PSUM"))

        R = {}
        # matmul operands first; modality v's side inputs immediately after
        # its chunks so the first dist op isn't starved
        for x in _MODS:
            nq = 1 if x == _MODS[0] else 4
            for k in range(2):
                te = const.tile([128, 2 * _M], f16, name=f"embsp_{x}_{k}")
                for q in range(nq):
                    w = 2 * _M // nq
                    nc.sync.dma_start(
                        te[:, q * w:(q + 1) * w],
                        din[f"embsp_{x}"][k * 128:(k + 1) * 128, q * w:(q + 1) * w])
                R[f"embsp_{x}_{k}"] = te
                ts = const.tile([128, 2 * _NL], f16, name=f"semsp_{x}_{k}")
                for q in range(nq):
                    w = 2 * _NL // nq
                    nc.sync.dma_start(
                        ts[:, q * w:(q + 1) * w],
                        din[f"semsp_{x}"][k * 128:(k + 1) * 128, q * w:(q + 1) * w])
                R[f"semsp_{x}_{k}"] = ts
            if x == _MODS[0]:
                tb = const.tile([128, _M], f32, name=f"e2br_v")
                nc.sync.dma_start(
                    tb[:], din["e2_v"][0:1, :].partition_broadcast(128))
                R["e2b_v"] = tb
                tf = const.tile([128, _NT], f32, name=f"f2Tr_v")
                nc.sync.dma_start(tf[:], din["f2T_v"][:])
                R["f2T_v"] = tf
            if x == _MODS[2]:
                for xx in _MODS[1:]:
                    tb2 = const.tile([128, _M], f32, name=f"e2br_{xx}")
                    nc.sync.dma_start(
                        tb2[:], din[f"e2_{xx}"][0:1, :].partition_broadcast(128))
                    R[f"e2b_{xx}"] = tb2
                    tf2 = const.tile([128, _NT], f32, name=f"f2Tr_{xx}")
                    nc.sync.dma_start(tf2[:], din[f"f2T_{xx}"][:])
                    R[f"f2T_{xx}"] = tf2
        for x in _MODS:
            # h1s = h1 * 2^-12 derived on device (bit-exact incl. subnormals)
            for k in range(2):
                th = const.tile([128, _NL], f16, name=f"h1s_{x}_{k}")
                nc.gpsimd.tensor_scalar(
                    th[:], R[f"semsp_{x}_{k}"][:, 0:_NL], 2.0 ** -12, None, ALU.mult)
                R[f"h1s_{x}_{k}"] = th
            tsem = const.tile([128, _NT, _D], f32, name=f"semr_{x}")
            nc.sync.dma_start(tsem[:], din[f"sem_{x}"].rearrange("(j p) d -> p j d", p=128))
            R[f"sem_{x}"] = tsem
        idxacc = const.tile([128, 3 * _NT, 8], u32, name="idxacc")
        phacc = const.tile([_BL, 3 * _M], f32, name="phacc")
        for xi0, x in enumerate(_MODS):
            R[f"idxacc_{x}"] = idxacc[:, xi0 * _NT:(xi0 + 1) * _NT, :]

        for xi, x in enumerate(_MODS):
            o = xi * _D  # column slice of the gathered full row for this modality
            phps = psh.tile([_BL, _M], f32, name="phps")
            for jp in range(_NT // 2):
                qacc = work.tile([128, 2, _D], f32, name="qacc", bufs=3)
                # tile pair (2*jp, 2*jp+1) shares one [128, 2M] dist buffer so
                # the elementwise ACT passes amortize their fixed overhead
                dist = work.tile([128, 2 * _M], f32, name="dist", bufs=3)
                for jo in range(2):
                    j = 2 * jp + jo
                    # 2*dot via fp16 Dekker 3-pass, K=256 in 2 chunks
                    pd = psd.tile([128, _M], f32, name="pd")
                    for k in range(2):
                        ssp = R[f"semsp_{x}_{k}"]
                        esp = R[f"embsp_{x}_{k}"]
                        sl = slice(j * 128, (j + 1) * 128)
                        # h1s-dependent pass last: the first matmuls don't
                        # wait for the on-device h1s derivation at startup
                        passes = ((ssp[:, 0:_NL][:, sl], 0),
                                  (ssp[:, _NL:2 * _NL][:, sl], 0),
                                  (R[f"h1s_{x}_{k}"][:, sl], _M))
                        for pi, (lhs, eo) in enumerate(passes):
                            for n in range(2):
                                nc.tensor.matmul(
                                    pd[:, n * 512:(n + 1) * 512],
                                    lhs,
                                    esp[:, eo + n * 512: eo + (n + 1) * 512],
                                    start=(k == 0 and pi == 0),
                                    stop=(k == 1 and pi == 2),
                                )
                    # host ships -e2 and -f2, so stage0 = fl(-e2-f2) = -u and
                    # nd = fl(-u + 2*dot) = -dist with the reference's exact
                    # rounding sequence (round-to-nearest is sign-symmetric)
                    dh = dist[:, jo * _M:(jo + 1) * _M]
                    nc.vector.scalar_tensor_tensor(
                        dh, R[f"e2b_{x}"][:], R[f"f2T_{x}"][:, j:j + 1], pd[:],
                        ALU.add, ALU.add)
                    # argmax(-dist) = argmin(dist), first index on ties (both
                    # max8/max_index and jnp.argmin break ties to first)
                    mx8 = small.tile([128, 8], f32, name="mx8")
                    nc.vector.max(mx8[:], dh)
                    nc.vector.max_index(R[f"idxacc_{x}"][:, j, :], mx8[:], dh)
                # softmax numerator exp(-sqrt(dist)) over the pair; sqrt(dist)
                # is in [11,20] so the unstabilized exp stays in f32/bf16 range
                lnd = work.tile([128, 2 * _M], f32, name="lnd", bufs=2)
                nc.scalar.activation(lnd[:], dist[:], ACT.Ln, scale=-1.0)
                sqd = lnd
                nc.scalar.activation(sqd[:], lnd[:], ACT.Exp, scale=0.5)
                for jo in range(2):
                    j = 2 * jp + jo
                    ex = work.tile([128, _M], bf16, name="ex", bufs=3)
                    scol = small.tile([128, 1], f32, name="scol")
                    nc.scalar.activation(ex[:], sqd[:, jo * _M:(jo + 1) * _M],
                                         ACT.Exp, scale=-1.0, accum_out=scol[:])
                    # per-token weight 1/(T*rowsum) in this clip's lhsT column
                    rcol = small.tile([128, 1], f32, name="rcol")
                    nc.vector.reciprocal(rcol[:], scol[:])
                    rb4 = small.tile([128, _BL], bf16, name="rb4")
                    nc.gpsimd.memset(rb4[:], 0)
                    nc.vector.tensor_scalar(rb4[:, jp:jp + 1], rcol[:], 1.0 / _T, None, ALU.mult)
                    # defer the tiny pH matmuls in scheduler order so their
                    # rb4 dependency chain can't head-of-line-block PE's
                    # in-order stream between dist matmul bursts
                    with tc.high_priority(offset=-800):
                        for n in range(2):
                            nc.tensor.matmul(
                                phps[:, n * 512:(n + 1) * 512],
                                rb4[:],
                                ex[:, n * 512:(n + 1) * 512],
                                start=(j == 0),
                                stop=(j == _NT - 1),
                            )
                # gather both tiles' codebook rows (one indirect DMA each)
                gat = work.tile([128, 2, 3 * _D], f32, name="gat", bufs=2)
                for jo in range(2):
                    j = 2 * jp + jo
                    nc.gpsimd.indirect_dma_start(
                        gat[:, jo, :], None, emb[:],
                        bass.IndirectOffsetOnAxis(
                            ap=R[f"idxacc_{x}"][:, j, 0:1], axis=0),
                    )
                    nc.sync.dma_start(
                        dout[f"full_{x}"][j * 128:(j + 1) * 128, :], gat[:, jo, :])
                # straight-through quant: fl(sem + fl(q - sem)), both tiles at once
                semp = R[f"sem_{x}"][:, 2 * jp:2 * jp + 2, :]
                q1 = small.tile([128, 2, _D], f32, name="q1")
                nc.gpsimd.tensor_tensor(q1[:], gat[:, :, o:o + _D], semp, ALU.subtract)
                nc.gpsimd.tensor_tensor(qacc[:], q1[:], semp, ALU.add)
                nc.sync.dma_start(
                    dout[f"quant_{x}"][2 * jp * 128:(2 * jp + 2) * 128, :]
                    .rearrange("(jj p) d -> p jj d", p=128),
                    qacc[:])
            nc.scalar.copy(phacc[:, xi * _M:(xi + 1) * _M], phps[:])
        nc.sync.dma_start(dout["pH_all"][:], phacc[:])
        nc.sync.dma_start(dout["idx_all"][:], idxacc[:])

    # The act-table-load pass greedily picks the first set containing each
    # function, alternating natural_log <-> exp_and_others per tile (86 table
    # loads, ~110us). Hide Ln/Exp from every set except the one that has both
    # so the whole kernel runs off a single ACT table load.
    import concourse.bacc as bacc_mod
    orig_tables = bacc_mod.get_activation_tables

    def _pinned_tables(arch):
        tabs = {k: set(v) for k, v in orig_tables(arch).items()}
        for name, funcs in tabs.items():
            if name != "natural_log_exp_and_others":
                funcs.discard(ACT.Exp)
                funcs.discard(ACT.Ln)
        return tabs

    bacc_mod.get_activation_tables = _pinned_tables
    try:
        nc.compile()
    finally:
        bacc_mod.get_activation_tables = orig_tables
    return nc


def _get_nc():
    if "nc" not in _CACHE:
        _CACHE["nc"] = _build_bass()
    return _CACHE["nc"]


def _split_lhs(st):
    """fp16 Dekker split of the [D, NL] f32 token matrix: [h1 | h2]."""
    h1 = st.astype(np.float16)
    h2 = (st - h1.astype(np.float32)).astype(np.float16)
    return np.concatenate([h1, h2], axis=1)


def _split_rhs(e):
    """fp16 split of the [D, M] f32 (2*emb^T) matrix: [e1 | (e-e1)*2^12]."""
    e1 = e.astype(np.float16)
    e2s = ((e - e1.astype(np.float32)) * 2.0 ** 12).astype(np.float16)
    return np.concatenate([e1, e2s], axis=1)


def kernel(audio_semantic=None, video_semantic=None, text_semantic=None,
           embedding=None, epoch=None, **_unused):
    import jax
    import jax.numpy as jnp
    from concourse.bass_utils import run_bass_kernel_spmd

    audio = np.asarray(audio_semantic, dtype=np.float32)
    video = np.asarray(video_semantic, dtype=np.float32)
    text = np.asarray(text_semantic, dtype=np.float32)
    emb = np.asarray(embedding, dtype=np.float32)

    sems = {"v": video, "a": audio, "t": text}
    offs = {"v": 0, "a": _D, "t": 2 * _D}

    cpu = jax.devices("cpu")[0]
    # e2/f2 must match the reference's XLA-CPU reduction bits exactly:
    # squares elementwise (bit-identical np vs jnp), sum via XLA reduce on CPU.
    e2 = {}
    f2 = {}
    with jax.default_device(cpu):
        for x in _MODS:
            sq = jnp.asarray((emb[:, offs[x]:offs[x] + _D]) ** 2)
            e2[x] = np.asarray(jnp.sum(sq, axis=1))
            fsq = jnp.asarray(sems[x].reshape(-1, _D) ** 2)
            f2[x] = np.asarray(jnp.sum(fsq, axis=1))

    shared = {"emb": emb}
    for x in _MODS:
        E = np.ascontiguousarray((2.0 * emb[:, offs[x]:offs[x] + _D]).T)
        shared[f"embsp_{x}"] = _split_rhs(E)
        shared[f"e2_{x}"] = np.ascontiguousarray(-e2[x][None, :])

    in_maps = []
    for c in range(_NC):
        m = dict(shared)
        for x in _MODS:
            flat_c = np.ascontiguousarray(
                sems[x].reshape(_B * _T, _D)[c * _NL:(c + 1) * _NL])
            m[f"sem_{x}"] = flat_c
            m[f"semsp_{x}"] = _split_lhs(np.ascontiguousarray(flat_c.T))
            f2c = f2[x][c * _NL:(c + 1) * _NL]
            m[f"f2T_{x}"] = np.ascontiguousarray(-f2c.reshape(_NT, 128).T)
        in_maps.append(m)

    nc = _get_nc()
    import os
    trace = bool(int(os.environ.get("BASS_KERNEL_TRACE", "0")))
    res = run_bass_kernel_spmd(nc, in_maps, list(range(_NC)), trace=trace)
    _CACHE["last_results"] = res

    full, quant, idx, pH = {}, {}, {}, {}
    for x in _MODS:
        full[x] = np.concatenate(
            [res.results[c][f"full_{x}"] for c in range(_NC)], axis=0
        ).reshape(_B, _T, 3 * _D)
        quant[x] = np.concatenate(
            [res.results[c][f"quant_{x}"] for c in range(_NC)], axis=0
        ).reshape(_B, _T, _D)
        xi0 = _MODS.index(x)
        idx[x] = np.concatenate(
            [np.ascontiguousarray(
                res.results[c]["idx_all"][:, xi0 * _NT:(xi0 + 1) * _NT, 0].T).reshape(-1)
             for c in range(_NC)])
        pH[x] = np.concatenate(
            [res.results[c]["pH_all"][:, xi0 * _M:(xi0 + 1) * _M] for c in range(_NC)],
            axis=0)

    with jax.default_device(cpu):
        def lcmcm(pa, pb):
            pa = jnp.asarray(pa)
            pb = jnp.asarray(pb)
            S = pa @ jnp.log(pb.T + 1e-10) + pb @ jnp.log(pa.T + 1e-10)
            E = jnp.exp(S + jnp.max(-S))
            return -jnp.mean(jnp.log(jnp.diag(E) / (jnp.sum(E, axis=1) + 1e-5)))

        losses = np.asarray(jnp.stack([
            lcmcm(pH["a"], pH["v"]),
            lcmcm(pH["a"], pH["t"]),
            lcmcm(pH["t"], pH["v"]),
        ])).astype(np.float32)

    modes = {}
    for x in _MODS:
        rows = idx[x].reshape(_B, _T)
        modes[x] = np.array([np.bincount(r, minlength=_M).argmax() for r in rows])
    equal_num = np.int32(np.sum((modes["a"] == modes["v"]) & (modes["a"] == modes["t"])))

    return (full["v"], full["a"], full["t"],
            quant["v"], quant["a"], quant["t"],
            losses, equal_num)
